# revision 25
# baseline (speedup 1.0000x reference)
"""Single-step LSTM cell (B=131072, E=H=128) on 8 Trainium2 NeuronCores.

Strategy: pure data-parallel over the batch. Each core handles 16384 rows.
Host-side we pre-transpose each shard (x^T, h^T, c^T: [128, Bc]) so the
contraction dim (E/H) lands on SBUF partitions — no on-chip transposes.
Gates are computed in transposed layout [H, batch] with the small weight
matrices stationary on the PE and bf16 matmuls streaming 512-column batch
chunks into PSUM (fp32 accumulate). Bias enters as a K=128 matmul of
broadcast(b/128) against an all-ones moving operand (a K=1 rank-1 matmul
stalls the PE pipeline). All four gates go through one fused sigmoid ACT
op — tanh(z) = 2*sigmoid(2z)-1 with the 2x folded into Wc/Uc/bc on the
host and the affine fixup done on DVE. Elementwise runs on DVE in bf16
(2x mode); tanh(c) is batched over two quarters on ACT. The c path
(c_prev in, c out) stays fp32 for accuracy; x/h/weights move as bf16,
which also halves their HBM traffic. A short burst of warmup matmuls
overlaps the first DMA so the PE clock (HAM) is at 2.4 GHz before real
work arrives. Steady state is ACT-bound with every engine near its
floor: ~109 us vs a ~117 us fp32 memory roofline per core.
"""

import numpy as np

B, E, H = 131072, 128, 128
NCORES = 8
BC = B // NCORES        # 16384 batch rows per core
CHUNK = 4096            # DMA chunk (batch cols per 1 MiB bf16 transfer)
QUART = 512             # compute chunk (batch cols per matmul group)
G4 = 4 * H              # 512, four gates concatenated

_CACHE = {}


def _build_nc():
    import concourse.bacc as bacc
    import concourse.mybir as mybir
    import concourse.tile as tile

    f32 = mybir.dt.float32
    bf = mybir.dt.bfloat16
    AF = mybir.ActivationFunctionType
    ALU = mybir.AluOpType

    nc = bacc.Bacc("TRN2", target_bir_lowering=False, debug=False,
                   num_devices=NCORES)

    xT = nc.dram_tensor("xT", [E, BC], bf, kind="ExternalInput").ap()
    hT = nc.dram_tensor("hT", [H, BC], bf, kind="ExternalInput").ap()
    cT = nc.dram_tensor("cT", [H, BC], f32, kind="ExternalInput").ap()
    W = nc.dram_tensor("W", [E, G4], bf, kind="ExternalInput").ap()
    U = nc.dram_tensor("U", [H, G4], bf, kind="ExternalInput").ap()
    bias = nc.dram_tensor("b", [E, G4], bf, kind="ExternalInput").ap()
    ones = nc.dram_tensor("ones", [E, QUART], bf, kind="ExternalInput").ap()
    hT_out = nc.dram_tensor("hT_out", [H, BC], bf, kind="ExternalOutput").ap()
    cT_out = nc.dram_tensor("cT_out", [H, BC], f32, kind="ExternalOutput").ap()

    n_chunks = BC // CHUNK
    n_quart = CHUNK // QUART

    with tile.TileContext(nc) as tc:
        with tc.tile_pool(name="cst", bufs=1) as cst, \
             tc.tile_pool(name="xin", bufs=3) as xin, \
             tc.tile_pool(name="hin", bufs=3) as hin, \
             tc.tile_pool(name="cin", bufs=3) as cin, \
             tc.tile_pool(name="hout", bufs=2) as hout, \
             tc.tile_pool(name="cout", bufs=2) as cout, \
             tc.tile_pool(name="work", bufs=4) as work, \
             tc.tile_pool(name="ps", bufs=2, space="PSUM") as ps:

            W_sb = cst.tile([E, G4], bf)
            U_sb = cst.tile([H, G4], bf)
            b_sb = cst.tile([E, G4], bf)
            ones_sb = cst.tile([E, QUART], bf)
            nc.sync.dma_start(out=b_sb[:], in_=bias)
            nc.sync.dma_start(out=ones_sb[:], in_=ones)

            # warm the PE (HAM clock ramp) while the first chunk loads
            wsrc = cst.tile([E, QUART], bf, name="wsrc")
            nc.vector.memset(wsrc[:], 1.0)
            warm = ps.tile([H, 4 * QUART], f32, name="warm", tag="gates")
            for _ in range(7):
                nc.tensor.matmul(warm[:, 0:QUART], wsrc[:, 0:H], wsrc[:],
                                 start=True, stop=True)

            for ch in range(n_chunks):
                off = ch * CHUNK
                x_sb = xin.tile([E, CHUNK], bf)
                h_sb = hin.tile([H, CHUNK], bf)
                c_sb = cin.tile([H, CHUNK], f32)
                # first chunk loads at quarter granularity so the first
                # matmul group's inputs land as early as possible
                nparts = 4 if ch == 0 else 2
                PC = CHUNK // nparts
                for hf in range(nparts):
                    o2, o3 = hf * PC, off + hf * PC
                    nc.sync.dma_start(out=x_sb[:, o2:o2 + PC],
                                      in_=xT[:, o3:o3 + PC])
                    nc.sync.dma_start(out=h_sb[:, o2:o2 + PC],
                                      in_=hT[:, o3:o3 + PC])
                    if ch == 0 and hf == 0:
                        # weights can land after the first data quarter
                        nc.sync.dma_start(out=W_sb[:], in_=W)
                        nc.sync.dma_start(out=U_sb[:], in_=U)
                    if ch > 0:
                        nc.sync.dma_start(out=c_sb[:, o2:o2 + PC],
                                          in_=cT[:, o3:o3 + PC])
                if ch == 0:
                    for hf in range(2):
                        o2 = hf * (CHUNK // 2)
                        nc.sync.dma_start(out=c_sb[:, o2:o2 + CHUNK // 2],
                                          in_=cT[:, off + o2:
                                               off + o2 + CHUNK // 2])

                ho_sb = hout.tile([H, CHUNK], bf)
                co_sb = cout.tile([H, CHUNK], f32)

                og_keep = {}
                for q in range(n_quart):
                    qo = q * QUART
                    xq = x_sb[:, qo:qo + QUART]
                    hq = h_sb[:, qo:qo + QUART]
                    cq = c_sb[:, qo:qo + QUART]

                    # gates^T in PSUM: 4 banks, one per gate (i|f|o|c~),
                    # with the c~ gate pre-scaled by 2 (host-side)
                    gates = ps.tile([H, 4 * QUART], f32)
                    for g in range(4):
                        gs = gates[:, g * QUART:(g + 1) * QUART]
                        bg = b_sb[:, g * H:(g + 1) * H]
                        Wg = W_sb[:, g * H:(g + 1) * H]
                        Ug = U_sb[:, g * H:(g + 1) * H]
                        nc.tensor.matmul(gs, bg, ones_sb[:],
                                         start=True, stop=False)
                        nc.tensor.matmul(gs, Wg, xq, start=False, stop=False)
                        nc.tensor.matmul(gs, Ug, hq, start=False, stop=True)

                    # one fused sigmoid over all 4 gates; for c~ this is
                    # sigmoid(2z) thanks to the host-side 2x fold
                    sig = work.tile([H, 4 * QUART], bf, tag="sig", bufs=6)
                    nc.scalar.activation(sig[:], gates[:], AF.Sigmoid)

                    i_g = sig[:, 0:QUART]
                    f_g = sig[:, QUART:2 * QUART]
                    o_g = sig[:, 2 * QUART:3 * QUART]
                    s_c = sig[:, 3 * QUART:4 * QUART]

                    # c~ = tanh(z) = 2*sigmoid(2z) - 1  (affine fixup)
                    ctl = work.tile([H, QUART], bf, tag="ctl")
                    nc.vector.tensor_scalar(out=ctl[:], in0=s_c,
                                            scalar1=2.0, scalar2=-1.0,
                                            op0=ALU.mult, op1=ALU.add)

                    m1 = work.tile([H, QUART], bf, tag="m1")
                    m2 = work.tile([H, QUART], bf, tag="m2")
                    nc.vector.tensor_mul(out=m1[:], in0=f_g, in1=cq)
                    nc.vector.tensor_mul(out=m2[:], in0=i_g, in1=ctl[:])
                    c_new = co_sb[:, qo:qo + QUART]
                    nc.vector.tensor_add(out=c_new, in0=m1[:], in1=m2[:])

                    # batch tanh over quarter pairs, except at the very
                    # end where per-quarter ops drain the pipeline sooner
                    last2 = (ch == n_chunks - 1 and q >= n_quart - 2)
                    og_keep[q % 2] = o_g
                    if last2:
                        tc_sb = work.tile([H, QUART], bf, tag="tc")
                        nc.scalar.activation(tc_sb[:], c_new, AF.Tanh)
                        nc.vector.tensor_mul(out=ho_sb[:, qo:qo + QUART],
                                             in0=o_g, in1=tc_sb[:])
                    elif q % 2 == 1:
                        lo = (q - 1) * QUART
                        tc_sb = work.tile([H, 2 * QUART], bf, tag="tc")
                        nc.scalar.activation(tc_sb[:],
                                             co_sb[:, lo:lo + 2 * QUART],
                                             AF.Tanh)
                        for qq in range(2):
                            nc.vector.tensor_mul(
                                out=ho_sb[:, lo + qq * QUART:
                                          lo + (qq + 1) * QUART],
                                in0=og_keep[qq],
                                in1=tc_sb[:, qq * QUART:(qq + 1) * QUART])

                nparts = 4 if ch == n_chunks - 1 else 2
                PC = CHUNK // nparts
                for hf in range(nparts):
                    o2, o3 = hf * PC, off + hf * PC
                    nc.sync.dma_start(out=hT_out[:, o3:o3 + PC],
                                      in_=ho_sb[:, o2:o2 + PC])
                    nc.sync.dma_start(out=cT_out[:, o3:o3 + PC],
                                      in_=co_sb[:, o2:o2 + PC])

    nc.compile()
    return nc


def kernel(x, hidden_memory_tm1, Wi, Ui, bi, Wf, Uf, bf, Wog, Uog, bog,
           Wc, Uc, bc, _return_timing=False, _trace=False):
    from concourse.bass_utils import run_bass_kernel_spmd

    if "nc" not in _CACHE:
        _CACHE["nc"] = _build_nc()
    nc = _CACHE["nc"]

    import ml_dtypes
    bf16 = ml_dtypes.bfloat16
    x = np.asarray(x, np.float32)
    hm = np.asarray(hidden_memory_tm1, np.float32)
    # fold the tanh-via-sigmoid 2x into the c~ gate's weights and bias
    W = np.concatenate([Wi, Wf, Wog, 2.0 * Wc], axis=1).astype(bf16)
    U = np.concatenate([Ui, Uf, Uog, 2.0 * Uc], axis=1).astype(bf16)
    bcat = np.concatenate([bi, bf, bog, 2.0 * bc])
    # bias as a K=128 matmul: stationary holds b/128 broadcast over the
    # contraction dim, moving operand is all-ones; K=1 matmuls stall the PE.
    b = np.broadcast_to(bcat[None, :] / E, (E, G4)).astype(bf16)
    ones = np.ones((E, QUART), bf16)

    in_maps = []
    for c in range(NCORES):
        sl = slice(c * BC, (c + 1) * BC)
        in_maps.append({
            "xT": np.ascontiguousarray(x[sl].astype(bf16).T),
            "hT": np.ascontiguousarray(hm[0, sl].astype(bf16).T),
            "cT": np.ascontiguousarray(hm[1, sl].T),
            "W": W, "U": U, "b": b, "ones": ones,
        })

    res = run_bass_kernel_spmd(nc, in_maps, core_ids=list(range(NCORES)),
                               trace=_trace)

    h = np.concatenate(
        [res.results[c]["hT_out"].T.astype(np.float32) for c in range(NCORES)], 0)
    cc = np.concatenate(
        [res.results[c]["cT_out"].T.astype(np.float32) for c in range(NCORES)], 0)
    out = np.stack([h, cc])
    if _return_timing:
        return out, res
    return out


# revision 26
# speedup vs baseline: 1.0097x; 1.0097x over previous
"""Single-step LSTM cell (B=131072, E=H=128) on 8 Trainium2 NeuronCores.

Strategy: pure data-parallel over the batch. Each core handles 16384 rows.
Host-side we pre-transpose each shard (x^T, h^T, c^T: [128, Bc]) so the
contraction dim (E/H) lands on SBUF partitions — no on-chip transposes.
Gates are computed in transposed layout [H, batch] with the small weight
matrices stationary on the PE and bf16 matmuls streaming 512-column batch
chunks into PSUM (fp32 accumulate). Bias enters as a K=128 matmul of
broadcast(b/128) against an all-ones moving operand (a K=1 rank-1 matmul
stalls the PE pipeline). All four gates go through one fused sigmoid ACT
op — tanh(z) = 2*sigmoid(2z)-1 with the 2x folded into Wc/Uc/bc on the
host and the affine fixup done on DVE. Elementwise runs on DVE in bf16
(2x mode); tanh(c) is batched over two quarters on ACT. The c path
(c_prev in, c out) stays fp32 for accuracy; x/h/weights move as bf16,
which also halves their HBM traffic. A short burst of warmup matmuls
overlaps the first DMA so the PE clock (HAM) is at 2.4 GHz before real
work arrives. Steady state is ACT-bound with every engine near its
floor: ~109 us vs a ~117 us fp32 memory roofline per core.
"""

import numpy as np

B, E, H = 131072, 128, 128
NCORES = 8
BC = B // NCORES        # 16384 batch rows per core
CHUNK = 4096            # DMA chunk (batch cols per 1 MiB bf16 transfer)
QUART = 512             # compute chunk (batch cols per matmul group)
G4 = 4 * H              # 512, four gates concatenated

_CACHE = {}


def _build_nc():
    import concourse.bacc as bacc
    import concourse.mybir as mybir
    import concourse.tile as tile

    f32 = mybir.dt.float32
    bf = mybir.dt.bfloat16
    AF = mybir.ActivationFunctionType
    ALU = mybir.AluOpType

    nc = bacc.Bacc("TRN2", target_bir_lowering=False, debug=False,
                   num_devices=NCORES)

    xT = nc.dram_tensor("xT", [E, BC], bf, kind="ExternalInput").ap()
    hT = nc.dram_tensor("hT", [H, BC], bf, kind="ExternalInput").ap()
    cT = nc.dram_tensor("cT", [H, BC], f32, kind="ExternalInput").ap()
    W = nc.dram_tensor("W", [E, G4], bf, kind="ExternalInput").ap()
    U = nc.dram_tensor("U", [H, G4], bf, kind="ExternalInput").ap()
    bias = nc.dram_tensor("b", [E, G4], bf, kind="ExternalInput").ap()
    ones = nc.dram_tensor("ones", [E, QUART], bf, kind="ExternalInput").ap()
    hT_out = nc.dram_tensor("hT_out", [H, BC], bf, kind="ExternalOutput").ap()
    cT_out = nc.dram_tensor("cT_out", [H, BC], f32, kind="ExternalOutput").ap()

    n_chunks = BC // CHUNK
    n_quart = CHUNK // QUART

    with tile.TileContext(nc) as tc:
        with tc.tile_pool(name="cst", bufs=1) as cst, \
             tc.tile_pool(name="xin", bufs=3) as xin, \
             tc.tile_pool(name="hin", bufs=3) as hin, \
             tc.tile_pool(name="cin", bufs=3) as cin, \
             tc.tile_pool(name="hout", bufs=2) as hout, \
             tc.tile_pool(name="cout", bufs=2) as cout, \
             tc.tile_pool(name="work", bufs=4) as work, \
             tc.tile_pool(name="ps", bufs=2, space="PSUM") as ps:

            W_sb = cst.tile([E, G4], bf)
            U_sb = cst.tile([H, G4], bf)
            b_sb = cst.tile([E, G4], bf)
            ones_sb = cst.tile([E, QUART], bf)
            nc.sync.dma_start(out=b_sb[:], in_=bias)
            nc.sync.dma_start(out=ones_sb[:], in_=ones)

            # warm the PE (HAM clock ramp) while the first chunk loads
            wsrc = cst.tile([E, QUART], bf, name="wsrc")
            nc.vector.memset(wsrc[:], 1.0)
            warm = ps.tile([H, 4 * QUART], f32, name="warm", tag="gates")
            for _ in range(7):
                nc.tensor.matmul(warm[:, 0:QUART], wsrc[:, 0:H], wsrc[:],
                                 start=True, stop=True)

            for ch in range(n_chunks):
                off = ch * CHUNK
                x_sb = xin.tile([E, CHUNK], bf)
                h_sb = hin.tile([H, CHUNK], bf)
                c_sb = cin.tile([H, CHUNK], f32)
                # first chunk loads at quarter granularity so the first
                # matmul group's inputs land as early as possible
                nparts = 4 if ch == 0 else 2
                PC = CHUNK // nparts
                for hf in range(nparts):
                    o2, o3 = hf * PC, off + hf * PC
                    nc.sync.dma_start(out=x_sb[:, o2:o2 + PC],
                                      in_=xT[:, o3:o3 + PC])
                    nc.sync.dma_start(out=h_sb[:, o2:o2 + PC],
                                      in_=hT[:, o3:o3 + PC])
                    if ch == 0 and hf == 0:
                        # weights can land after the first data quarter
                        nc.sync.dma_start(out=W_sb[:], in_=W)
                        nc.sync.dma_start(out=U_sb[:], in_=U)
                    if ch > 0:
                        nc.sync.dma_start(out=c_sb[:, o2:o2 + PC],
                                          in_=cT[:, o3:o3 + PC])
                if ch == 0:
                    for hf in range(2):
                        o2 = hf * (CHUNK // 2)
                        nc.sync.dma_start(out=c_sb[:, o2:o2 + CHUNK // 2],
                                          in_=cT[:, off + o2:
                                               off + o2 + CHUNK // 2])

                ho_sb = hout.tile([H, CHUNK], bf)
                co_sb = cout.tile([H, CHUNK], f32)

                og_keep = {}
                for q in range(n_quart):
                    qo = q * QUART
                    xq = x_sb[:, qo:qo + QUART]
                    hq = h_sb[:, qo:qo + QUART]
                    cq = c_sb[:, qo:qo + QUART]

                    # gates^T in PSUM: 4 banks, one per gate (i|f|o|c~),
                    # with the c~ gate pre-scaled by 2 (host-side)
                    gates = ps.tile([H, 4 * QUART], f32)
                    for g in range(4):
                        gs = gates[:, g * QUART:(g + 1) * QUART]
                        bg = b_sb[:, g * H:(g + 1) * H]
                        Wg = W_sb[:, g * H:(g + 1) * H]
                        Ug = U_sb[:, g * H:(g + 1) * H]
                        nc.tensor.matmul(gs, bg, ones_sb[:],
                                         start=True, stop=False)
                        nc.tensor.matmul(gs, Wg, xq, start=False, stop=False)
                        nc.tensor.matmul(gs, Ug, hq, start=False, stop=True)

                    # one fused sigmoid over all 4 gates; for c~ this is
                    # sigmoid(2z) thanks to the host-side 2x fold
                    sig = work.tile([H, 4 * QUART], bf, tag="sig", bufs=6)
                    nc.scalar.activation(sig[:], gates[:], AF.Sigmoid)

                    i_g = sig[:, 0:QUART]
                    f_g = sig[:, QUART:2 * QUART]
                    o_g = sig[:, 2 * QUART:3 * QUART]
                    s_c = sig[:, 3 * QUART:4 * QUART]

                    # c~ = tanh(z) = 2*sigmoid(2z) - 1  (affine fixup)
                    ctl = work.tile([H, QUART], bf, tag="ctl")
                    nc.vector.tensor_scalar(out=ctl[:], in0=s_c,
                                            scalar1=2.0, scalar2=-1.0,
                                            op0=ALU.mult, op1=ALU.add)

                    m1 = work.tile([H, QUART], bf, tag="m1")
                    m2 = work.tile([H, QUART], bf, tag="m2")
                    nc.vector.tensor_mul(out=m1[:], in0=f_g, in1=cq)
                    nc.vector.tensor_mul(out=m2[:], in0=i_g, in1=ctl[:])
                    c_new = co_sb[:, qo:qo + QUART]
                    nc.vector.tensor_add(out=c_new, in0=m1[:], in1=m2[:])

                    og_keep[q % 2] = o_g
                    if q % 2 == 1:
                        lo = (q - 1) * QUART
                        tc_sb = work.tile([H, 2 * QUART], bf, tag="tc")
                        nc.scalar.activation(tc_sb[:],
                                             co_sb[:, lo:lo + 2 * QUART],
                                             AF.Tanh)
                        for qq in range(2):
                            nc.vector.tensor_mul(
                                out=ho_sb[:, lo + qq * QUART:
                                          lo + (qq + 1) * QUART],
                                in0=og_keep[qq],
                                in1=tc_sb[:, qq * QUART:(qq + 1) * QUART])

                nparts = 4 if ch == n_chunks - 1 else 2
                PC = CHUNK // nparts
                for hf in range(nparts):
                    o2, o3 = hf * PC, off + hf * PC
                    nc.sync.dma_start(out=hT_out[:, o3:o3 + PC],
                                      in_=ho_sb[:, o2:o2 + PC])
                    nc.sync.dma_start(out=cT_out[:, o3:o3 + PC],
                                      in_=co_sb[:, o2:o2 + PC])

    nc.compile()
    return nc


def kernel(x, hidden_memory_tm1, Wi, Ui, bi, Wf, Uf, bf, Wog, Uog, bog,
           Wc, Uc, bc, _return_timing=False, _trace=False):
    from concourse.bass_utils import run_bass_kernel_spmd

    if "nc" not in _CACHE:
        _CACHE["nc"] = _build_nc()
    nc = _CACHE["nc"]

    import ml_dtypes
    bf16 = ml_dtypes.bfloat16
    x = np.asarray(x, np.float32)
    hm = np.asarray(hidden_memory_tm1, np.float32)
    # fold the tanh-via-sigmoid 2x into the c~ gate's weights and bias
    W = np.concatenate([Wi, Wf, Wog, 2.0 * Wc], axis=1).astype(bf16)
    U = np.concatenate([Ui, Uf, Uog, 2.0 * Uc], axis=1).astype(bf16)
    bcat = np.concatenate([bi, bf, bog, 2.0 * bc])
    # bias as a K=128 matmul: stationary holds b/128 broadcast over the
    # contraction dim, moving operand is all-ones; K=1 matmuls stall the PE.
    b = np.broadcast_to(bcat[None, :] / E, (E, G4)).astype(bf16)
    ones = np.ones((E, QUART), bf16)

    in_maps = []
    for c in range(NCORES):
        sl = slice(c * BC, (c + 1) * BC)
        in_maps.append({
            "xT": np.ascontiguousarray(x[sl].astype(bf16).T),
            "hT": np.ascontiguousarray(hm[0, sl].astype(bf16).T),
            "cT": np.ascontiguousarray(hm[1, sl].T),
            "W": W, "U": U, "b": b, "ones": ones,
        })

    res = run_bass_kernel_spmd(nc, in_maps, core_ids=list(range(NCORES)),
                               trace=_trace)

    h = np.concatenate(
        [res.results[c]["hT_out"].T.astype(np.float32) for c in range(NCORES)], 0)
    cc = np.concatenate(
        [res.results[c]["cT_out"].T.astype(np.float32) for c in range(NCORES)], 0)
    out = np.stack([h, cc])
    if _return_timing:
        return out, res
    return out
